# revision 36
# baseline (speedup 1.0000x reference)
"""Duration-based length regulation (KittenTTS LengthRegulator) on 8 trn2 NeuronCores.

For each batch b (one per core): phoneme t's feature row is repeated
clamp(durations[b,t],1) times along the frame axis; frames are zero-padded to
MAX_LEN = T*15 (padding rows rely on the runner's pre-zeroed output buffers).

Phonemes map to (partition, block) as t = 4p + j, so ONE feature DMA lands
all 512 rows with contiguous 8KB-per-partition descriptors (3x the delivery
rate of row-per-partition 2KB descriptors).

Per-core pipeline (batch-parallel across 8 cores):
  1. Loads: durations (sync, first - heads the offset critical path),
     features in one DMA into a [128, 4*512] landing tile (sync), constant
     tables on the scalar engine's HWDGE queue.
  2. Inclusive cumsum of clamp(dur,1) over flat order t = 4p+j: free-dim
     row scan + ONE PE matmul (strict-lower-triangular ones, bf16 exact for
     these small integers) for the partition-dim prefix of row sums.
  3. Offsets for all four scatter passes (s=8,4,2,1) in one [128,16]
     vectorized block on DVE: off = exc + (dur & -(2s)), pushed OOB
     (>= 1<<20) unless (dur & s). DVE replication copies carry a scheduling
     fence (tile_wait_until) so the greedy per-engine scheduler cannot slot
     a long copy into an offset-chain semaphore stall.
  4. Row replication x8 into per-block [128, 8*512] tiles (kept at 16KB per
     partition - bigger tiles lose the DVE 4x perf mode): DVE doubling
     copies for blocks 0,1 and block 3's low half, ACT stride-0
     broadcast-read ops for block 2 and block 3's high half.
  5. 16 indirect scatter DMAs (SWDGE emission is ~1.4us each and
     one-offset-per-partition is a firmware limit, so 16 is minimal for the
     binary decomposition). The writes hit disjoint output rows, but the
     dependency tracker serializes same-tensor writes on COMPLETION
     (~5-7us each), so each block gets three output buffers (s1+s8 / s2 /
     s4); the only same-buffer pairs sit >=6 emission slots apart and the
     Pool engine never stalls. Emission order: s=1 first (reads the landing
     tile, whose only writers are the load DMAs), s=8 as soon as
     replication lands (overlaps later emissions instead of serializing
     into the tail), s=4 last (small completion tail). The host sums the
     twelve pre-zeroed row-disjoint buffers.
Each output row is written exactly once -> DMA write traffic ~= ragged size.
"""

import sys

import numpy as np

if "/opt/trn_rl_repo" not in sys.path:
    sys.path.insert(0, "/opt/trn_rl_repo")

B, T, D = 8, 512, 512
MAX_DUR = 15
MAX_LEN = T * MAX_DUR  # 7680
P = 128
NT = T // P  # 4 feature blocks
NCOPY = 8  # replicated copies per row (binary decomposition up to 15)
SBLK = [8, 4, 2, 1]  # scatter pass block sizes
OOB = 1 << 20  # pushed past bounds_check -> descriptor silently skipped

_CACHE = {}


def _build_nc():
    import ml_dtypes
    from concourse import bass, mybir
    from concourse.bacc import Bacc
    from concourse.tile import TileContext

    f32, i32, bf16 = mybir.dt.float32, mybir.dt.int32, mybir.dt.bfloat16
    Alu = mybir.AluOpType

    nc = Bacc()
    feats = nc.declare_dram_parameter("features", [T, D], f32, isOutput=False)
    durs_mat = nc.declare_dram_parameter("durations_t", [P, NT], i32, isOutput=False)
    # three output buffers per block (s1+s8 / s2 / s4): the dependency
    # tracker serializes writes to the same tensor on COMPLETION, so buffers
    # are assigned such that same-buffer emissions sit >=6 slots (~8.5us)
    # apart - beyond the ~5-7us DMA completion latency. The host sums the
    # pre-zeroed, row-disjoint buffers.
    outs = [
        nc.declare_dram_parameter(f"o{j}{h}", [MAX_LEN, D], f32, isOutput=True)
        for j in range(NT)
        for h in range(3)
    ]

    # NEFF-embedded constants:
    #  LO[:, 0:128] = Lstrict, L[k, m] = 1 iff k < m (exclusive partition prefix)
    lo_np = (np.arange(P)[:, None] < np.arange(P)[None, :]).astype(ml_dtypes.bfloat16)
    lo_const = nc.inline_tensor(np.ascontiguousarray(lo_np), name="lo_const")
    #  CT[:, 0:16] = -(2s) per wide column c = si*4+j; CT[:, 16:32] = s
    s_per_col = np.repeat(np.array(SBLK, np.int32), NT)  # [16]
    ct_np = np.broadcast_to(
        np.concatenate([-(2 * s_per_col), s_per_col])[None, :], (P, 2 * len(SBLK) * NT)
    ).astype(np.int32)
    ct_const = nc.inline_tensor(np.ascontiguousarray(ct_np), name="ct_const")

    NW = len(SBLK) * NT  # 16 wide columns

    with TileContext(nc) as tc:
        with tc.tile_pool(name="sbuf", bufs=1) as sb, tc.tile_pool(
            name="psum", bufs=1, space="PSUM"
        ) as pp:
            # --- loads --------------------------------------------------
            dur = sb.tile([P, NT], i32, tag="dur")
            nc.scalar.dma_start(out=dur[:], in_=durs_mat[:, :])
            lo = sb.tile([P, P], bf16, tag="lo")
            nc.sync.dma_start(out=lo[:], in_=lo_const[:, :])
            ct = sb.tile([P, 2 * NW], i32, tag="ct")
            nc.sync.dma_start(out=ct[:], in_=ct_const[:, :])
            # features split across both HWDGE queues (two 512KB DMAs with
            # contiguous 4KB-per-partition descriptors land ~2.5us sooner
            # than one 1MB transfer on a single queue)
            land01 = sb.tile([P, 2 * D], f32, tag="land01")
            land23 = sb.tile([P, 2 * D], f32, tag="land23")
            feats_r = feats[:, :].rearrange("(p j) d -> p (j d)", j=NT)
            nc.sync.dma_start(out=land01[:], in_=feats_r[:, 0 : 2 * D])
            nc.scalar.dma_start(out=land23[:], in_=feats_r[:, 2 * D : 4 * D])
            lands = [land01[:, 0:D], land01[:, D : 2 * D], land23[:, 0:D], land23[:, D : 2 * D]]
            rep = []
            for j in range(NT):
                rt = sb.tile([P, NCOPY * D], f32, tag=f"rep{j}")
                rep.append(rt)

            # --- cumsum over flat phoneme order t = 4p + j --------------
            nc.vector.tensor_scalar_max(out=dur[:], in0=dur[:], scalar1=1)
            einc = sb.tile([P, NT], i32, tag="einc")
            nc.vector.tensor_tensor_scan(
                out=einc[:], data0=dur[:], data1=dur[:], initial=0.0,
                op0=Alu.add, op1=Alu.bypass,
            )
            rs_h = sb.tile([P, 1], bf16, tag="rs_h")
            nc.vector.tensor_copy(out=rs_h[:], in_=einc[:, NT - 1 : NT])

            ps = pp.tile([P, 1], f32, tag="ps")
            nc.tensor.matmul(ps[:], lo[:, :], rs_h[:], start=True, stop=True)
            pfx = sb.tile([P, 1], i32, tag="pfx")
            nc.vector.tensor_copy(out=pfx[:], in_=ps[:])

            cum = sb.tile([P, NT], i32, tag="cum")
            nc.vector.tensor_tensor(
                out=cum[:], in0=einc[:], in1=pfx[:].to_broadcast([P, NT]), op=Alu.add
            )
            exc = sb.tile([P, NT], i32, tag="exc")
            nc.vector.tensor_tensor(out=exc[:], in0=cum[:], in1=dur[:], op=Alu.subtract)

            # --- widen dur/exc to [128, 16] (4 copies along s-passes) ---
            dur16 = sb.tile([P, NW], i32, tag="dur16")
            exc16 = sb.tile([P, NW], i32, tag="exc16")
            nc.vector.tensor_copy(out=dur16[:, 0:NT], in_=dur[:])
            nc.vector.tensor_copy(out=dur16[:, NT : 2 * NT], in_=dur[:])
            nc.vector.tensor_copy(out=dur16[:, 2 * NT : 4 * NT], in_=dur16[:, 0 : 2 * NT])
            nc.vector.tensor_copy(out=exc16[:, 0:NT], in_=exc[:])
            nc.vector.tensor_copy(out=exc16[:, NT : 2 * NT], in_=exc[:])
            nc.vector.tensor_copy(out=exc16[:, 2 * NT : 4 * NT], in_=exc16[:, 0 : 2 * NT])

            # --- scatter offsets, all passes at once --------------------
            offs = sb.tile([P, NW], i32, tag="offs")
            msk = sb.tile([P, NW], i32, tag="msk")
            nc.vector.tensor_tensor(out=offs[:], in0=dur16[:], in1=ct[:, 0:NW], op=Alu.bitwise_and)
            nc.vector.tensor_tensor(out=offs[:], in0=offs[:], in1=exc16[:], op=Alu.add)
            nc.vector.tensor_tensor(out=msk[:], in0=dur16[:], in1=ct[:, NW : 2 * NW], op=Alu.bitwise_and)
            nc.vector.tensor_scalar(
                out=msk[:], in0=msk[:], scalar1=0, scalar2=OOB, op0=Alu.is_equal, op1=Alu.mult
            )
            nc.vector.tensor_tensor(out=offs[:], in0=offs[:], in1=msk[:], op=Alu.add)

            # --- row replication ----------------------------------------
            def dve_block(j):
                with tc.tile_wait_until(0.012):
                    nc.vector.tensor_copy(out=rep[j][:, 0:D], in_=lands[j])
                for w in (1, 2, 4):
                    with tc.tile_wait_until(0.012):
                        nc.vector.tensor_copy(
                            out=rep[j][:, w * D : 2 * w * D], in_=rep[j][:, 0 : w * D]
                        )

            def bcast_ap(j, n):
                return rep[j][:, 0:D].rearrange("p (x d) -> p x d", x=1).to_broadcast(
                    [P, n, D]
                )

            def act_block(j):
                nc.scalar.copy(out=rep[j][:, 0:D], in_=lands[j])
                dst = rep[j][:, D : NCOPY * D].rearrange("p (x d) -> p x d", d=D)
                nc.scalar.copy(out=dst, in_=bcast_ap(j, NCOPY - 1))

            # copies balanced across engines (DVE ~1.65x faster per column):
            # DVE: blocks 0,1 + block 3's landing copy and low half;
            # ACT: block 2 + block 3's high half (both halves broadcast-read
            # from copy 0, so they don't chain on each other)
            dve_block(0)
            dve_block(1)
            act_block(2)
            with tc.tile_wait_until(0.012):
                nc.vector.tensor_copy(out=rep[3][:, 0:D], in_=lands[3])
            with tc.tile_wait_until(0.012):
                nc.vector.tensor_copy(
                    out=rep[3][:, D : 4 * D].rearrange("p (x d) -> p x d", d=D),
                    in_=bcast_ap(3, 3),
                )
            # the fence keeps this op AFTER block 2's broadcast in the ACT
            # stream - the scheduler's coarse per-engine counter thresholds
            # otherwise make block 2's scatters wait on this op too
            with tc.tile_wait_until(0.013):
                nc.scalar.copy(
                    out=rep[3][:, 4 * D : NCOPY * D].rearrange("p (x d) -> p x d", d=D),
                    in_=bcast_ap(3, NCOPY - 4),
                )

            breg = nc.gpsimd.to_reg(MAX_LEN - 1)

            # --- scatters, no critical sections. Order: s=1 first (needs
            # only the landing copy), the big s=8 transfers as soon as each
            # block's replication completes (so they overlap later emissions
            # instead of serializing into the completion tail), s=4 last
            # (small tail). Buffer pairing (s1+s8 -> h0, s2+s4 -> h1) keeps
            # same-buffer WAW pairs >=6 emission slots apart.
            order = (
                [(1, 2), (1, 3), (1, 0), (1, 1), (2, 0), (2, 1)]
                + [(8, 0), (8, 1), (2, 2), (2, 3), (8, 2), (8, 3)]
                + [(4, 0), (4, 1), (4, 2), (4, 3)]
            )
            for s_, j in order:
                si = SBLK.index(s_)
                c = si * NT + j
                # s=1 reads the landing tile (writers: just the two load
                # DMAs) - the dependency tracker is whole-tile, so reading
                # rep[j] would needlessly wait for ALL replication copies
                src = lands[j] if s_ == 1 else rep[j][:, 0 : s_ * D]
                h = {1: 0, 8: 0, 2: 1, 4: 2}[s_]
                nc.gpsimd.indirect_dma_start(
                    out=outs[3 * j + h][:, :],
                    out_offset=bass.IndirectOffsetOnAxis(
                        ap=offs[:, c : c + 1], axis=0
                    ),
                    in_=src,
                    in_offset=None,
                    bounds_check=breg,
                    oob_is_err=False,
                )

    nc.compile()
    return nc


def _get_nc():
    if "nc" not in _CACHE:
        _CACHE["nc"] = _build_nc()
    return _CACHE["nc"]


def _run(features, durations, trace=False):
    """features (B,T,D) f32, durations (B,T) i32 -> (out (B,MAX_LEN,D) f32, BassKernelResults)."""
    from concourse.bass_utils import run_bass_kernel_spmd

    nc = _get_nc()
    in_maps = []
    for b in range(B):
        dmat = np.ascontiguousarray(durations[b].reshape(P, NT))  # [P, NT], t = 4p+j
        in_maps.append(
            {
                "features": np.ascontiguousarray(features[b]),
                "durations_t": dmat,
            }
        )
    kwargs = {}
    if trace:
        kwargs = dict(trace=True, trace_cores=list(range(B)), stitch_traces=False)
    res = run_bass_kernel_spmd(nc, in_maps, core_ids=list(range(B)), **kwargs)
    # per-buffer outputs write disjoint rows of pre-zeroed memory: sum merges
    outs = np.stack(
        [
            sum(res.results[b][f"o{j}{h}"] for j in range(NT) for h in range(3))
            for b in range(B)
        ]
    )
    return outs.astype(np.float32, copy=False), res


def kernel(features, durations):
    features = np.asarray(features, dtype=np.float32)
    durations = np.asarray(durations, dtype=np.int32)
    outs, _ = _run(features, durations, trace=False)
    return outs


if __name__ == "__main__":
    feats = np.random.randn(B, T, D).astype(np.float32)
    durs = np.random.randint(0, 16, size=(B, T)).astype(np.int32)
    out = kernel(feats, durs)
    print("out", out.shape, out.dtype)
